# revision 60
# baseline (speedup 1.0000x reference)
"""Trainium2 Bass kernel: prototype-kNN CCE loss (nn_CCE_67190468378875).

Math: for each row b, the loss needs, per class, the min squared distance
over that class's 32 prototypes, evaluated at the target class (vt) and the
best non-target class (vw).  Equivalently per-proto score
nq[b,j] = 2 x_b.w_j - |w_j|^2; per-class MAX of nq gives -min d2 (+|x|^2).

Device work per core (batch-sharded 512 rows = 4 row-blocks of 128,
clusters replicated; prototype columns permuted plane-major; weights laid
out superblock-major in DRAM so every DMA chunk is contiguous/partition):

  psum[b, col] = 2 x_b . w_col + delta_col   (fp8 DoubleRow matmuls; the
      recentered bias delta = 512 - |w|^2 rides two sacrificial feature
      rows (64*u + v split), so no separate bias pass is needed).
      Superblock-PAIR schedule: per (pair q, row-block bb) 8 matmuls over
      two psum tiles; _dedup_ldweights() drops the per-matmul implicit
      LDWEIGHTS reloads (bass lowers every matmul into Ldweights+Matmult)
      down to 2 per pair.  Dependency-free warmup matmuls on a memset
      scratch bridge the DMA-gated head so the HAM clock gate (PE at
      1.2GHz until ~3.4us of sustained activity) releases before the real
      stream starts.
  consumers (seed-only, no accumulate chains or staging; host folds 20
      slot rows): even superblock of each pair: one ACT copies psum
      straight into 4 bf16 acc rows; odd superblock: one DVE TensorReduce
      over the 4 planes seeds one acc row (and frees the pair's lead psum
      tile fast).  Rows 0:15 ship after q=2, rows 15:20 after q=3
      (two-wave output DMA).
Host: input prep (fp8 cast, plane-major column permutation, feature-row
bias fold), 20-row max fold + vt/vw selection, final combine in f64.

This container's walrus build encodes at most ONE inline sync wait per TPB
instruction and rejects EVENT_SEMAPHORE_RANGE_CLEAR / INC_SWDGE_SEM ISA
ops and all gpsimd compute; _legalize_sync() post-processes the
Tile-scheduled module accordingly.
"""

import os
import numpy as np
import ml_dtypes
from contextlib import ExitStack

import concourse.bass as bass
import concourse.mybir as mybir
import concourse.tile as tile
from concourse.bass_utils import run_bass_kernel_spmd

B, C, P, F = 4096, 200, 32, 512
CP = C * P                  # 6400 prototypes
ALPHA, EPS = 5.0, 1e-8
N_CORES = 8
BLOC = B // N_CORES         # 512 rows per core
BB = BLOC // 128            # 4 row-blocks of 128
FC = F // 128               # 4 contraction chunks
T = 4                       # prototype planes per superblock
NSB = P // T                # 8 superblocks
SBW = C * T                 # 800 columns per superblock
PSUM_BUFS = int(os.environ.get("KPSB", "4"))  # psum pool tiles
PSW = int(os.environ.get("KPSW", "1024"))     # psum tile width (fp32 cols)
RECENTER = 512.0            # delta = RECENTER - |w|^2 rides the fold rows

_BF16 = mybir.dt.bfloat16
_F32 = mybir.dt.float32
_F8 = mybir.dt.float8e4

NROWS = 20                  # slot rows per row-block shipped to the host


def _emit(ctx, tc_ctx, io):
    """Superblock-pair schedule; see module docstring."""
    nc = tc_ctx.nc
    singles = ctx.enter_context(tc_ctx.tile_pool(name="singles", bufs=1))
    psum = ctx.enter_context(tc_ctx.tile_pool(name="psum", bufs=PSUM_BUFS,
                                              space="PSUM"))

    # weights laid out superblock-major so every DMA chunk is contiguous
    # per partition (one fat descriptor instead of FC strided stripes)
    wt_t = singles.tile([128, NSB, FC, SBW], _F8)
    xt_t = singles.tile([128, FC, BLOC], _F8)    # X^T (+ fold const rows)
    acc = singles.tile([128, BB, NROWS, C], _BF16)  # per-row-block slot rows
    wrm = singles.tile([128, 2, 128], _F8)       # HAM warmup scratch

    pm = mybir.MatmulPerfMode.DoubleRow

    # Input DMA dispatches cost ~0.65us of dispatcher-engine time each; put
    # wt superblock 0 first (it gates the first real matmul) and xt on the
    # scalar engine so the two leading feeds start in parallel.
    wt_in = io["wt"][:, :].rearrange("p (s fc j) -> p s fc j", s=NSB, fc=FC)
    nc.sync.dma_start(out=wt_t[:, 0:1], in_=wt_in[:, 0:1])
    nc.scalar.dma_start(
        out=xt_t[:, :, :],
        in_=io["xt"][:, :].rearrange("p (fc b) -> p fc b", fc=FC))
    for s0, s1 in ((1, 2), (2, 3), (3, 4), (4, 6), (6, 8)):
        nc.sync.dma_start(out=wt_t[:, s0:s1], in_=wt_in[:, s0:s1])

    # HAM warmup: the PE clock-gate sits at 1.2GHz until ~3.4us of sustained
    # matmul activity.  Spin dependency-free garbage matmuls during the DMA
    # head so the real stream runs at 2.4GHz from its first instruction.
    n_warm = int(os.environ.get("KWARM", "36"))
    if n_warm:
        nc.vector.memset(wrm[:, :, :], 0.0)
        pw = psum.tile([128, PSW], _F32, tag="ps")
        for _ in range(n_warm):
            nc.tensor.matmul(pw[:, 0:128], wrm[:, :, 0:128], wrm[:, :, :],
                             start=True, stop=True, perf_mode=pm,
                             skip_group_check=True)

    NQ = NSB // 2            # 4 superblock pairs
    order = [(q, bb) for q in range(NQ - 1) for bb in range(BB)]
    order += [(NQ - 1, bb) for bb in range(BB - 1, -1, -1)]

    # seed-only consumer plan: every drain seeds its own slot rows in acc
    # (no accumulate chains, no staging, no TT merges at all), host folds
    # the NROWS rows per row-block.  Row layout is wave-ordered so rows
    # 0:15 are final after the q=2 pair (shipped mid-stream) and only rows
    # 15:20 ride the tail:
    #   h0 (even sb):  ACT copies psum straight into 4 acc rows
    #                  (q0->0:4, q1->4:8, q2->8:12, q3->15:19)
    #   h1 (odd sb):   TR over the 4 planes seeds one acc row
    #                  (q0->12, q1->13, q2->14, q3->19)
    def consume0(q, ps, bb):     # h0 chain: ACT seeds 4 rows directly
        r0 = {0: 0, 1: 4, 2: 8, 3: 15}[q]
        psv = ps[:, 0:SBW].rearrange("p (c t) -> p t c", t=T)
        nc.scalar.activation(acc[:, bb, r0:r0 + T, :], psv,
                             mybir.ActivationFunctionType.Copy)

    def consume1(q, ps, bb):     # h1 chain: direct DVE reduce, frees psum
        row = {0: 12, 1: 13, 2: 14, 3: 19}[q]
        rin = ps[:, 0:SBW].rearrange("p (c t) -> p c t", t=T)
        nc.vector.tensor_reduce(
            out=acc[:, bb, row, :], in_=rin,
            axis=mybir.AxisListType.X, op=mybir.AluOpType.max)

    for q, bb in order:
        s0, s1 = 2 * q, 2 * q + 1
        ps0 = psum.tile([128, PSW], _F32, tag="ps")
        ps1 = psum.tile([128, PSW], _F32, tag="ps")
        rows = slice(bb * 128, (bb + 1) * 128)
        for pi in range(2):
            # narrow (288-col) matmuls first, wide (512) last: the trailing
            # 512 fully hides the next group's LDWEIGHTS (126ns) while a
            # trailing 288 (120ns stream) would expose it
            for p0, p1 in ((512, SBW), (0, 512)):
                for s, ps in ((s0, ps0), (s1, ps1)):
                    nc.tensor.matmul(
                        ps[:, p0:p1],
                        xt_t[:, 2 * pi:2 * pi + 2, rows],
                        wt_t[:, s, 2 * pi:2 * pi + 2, p0:p1],
                        start=(pi == 0), stop=(pi == 1), perf_mode=pm)

        consume1(q, ps0, bb)   # TR drains the pair's FIRST psum tile: the
        consume0(q, ps1, bb)   # next-next pair's lead MMs see a fast free

        if q == NQ - 2:
            nc.sync.dma_start(
                out=io["accq"][:, bb * NROWS * C:bb * NROWS * C + 15 * C],
                in_=acc[:, bb, 0:15, :].rearrange("p r c -> p (r c)"))
        elif q == NQ - 1:
            nc.sync.dma_start(
                out=io["accq"][:, bb * NROWS * C + 15 * C:
                               (bb + 1) * NROWS * C],
                in_=acc[:, bb, 15:20, :].rearrange("p r c -> p (r c)"))


_RANGE_CLEAR_OPCODE = 176


def _dedup_ldweights(nc):
    """Drop InstLdweights whose stationary AP matches the previous Ldweights
    on the PE stream (bass lowers every matmul into Ldweights + Matmult;
    consecutive matmuls sharing lhsT reload identical weights for nothing --
    each reload is ~126ns of PE time plus sequencer pressure).

    A dropped Ldweights' waits and sem updates migrate to the next kept PE
    instruction so cross-engine vector clocks stay intact.
    """
    for fn in nc.m.functions:
        for blk in fn.blocks:
            last_key = None
            pend_waits, pend_ups = [], []
            out = []
            for ins in blk.instructions:
                tn = type(ins).__name__
                if getattr(ins, "engine", None) != mybir.EngineType.PE:
                    out.append(ins)
                    continue
                if tn == "InstLdweights":
                    key = (str(ins.ins[0]), str(ins.perf_mode),
                           str(getattr(ins, "tile_position", None)))
                    si = ins.sync_info
                    if key == last_key:
                        if si is not None:
                            pend_waits += list(si.on_wait)
                            pend_ups += list(si.on_update)
                        continue
                    last_key = key
                if pend_waits or pend_ups:
                    si = ins.sync_info
                    waits = list(si.on_wait) if si else []
                    ups = list(si.on_update) if si else []
                    # merge duplicate sem-inc updates by summing values
                    for u in pend_ups:
                        for v in ups:
                            if (u.sync_type == v.sync_type
                                    and getattr(u, "id", None) == getattr(v, "id", None)
                                    and u.update_mode == v.update_mode == "sem-inc"):
                                v.update_value += u.update_value
                                break
                        else:
                            ups.append(u)
                    ins.sync_info = mybir.SyncInfo(
                        on_wait=pend_waits + waits, on_update=ups)
                    pend_waits, pend_ups = [], []
                out.append(ins)
            assert not pend_waits and not pend_ups, (
                "dangling sync from dropped trailing Ldweights")
            if hasattr(blk, "set_instructions"):
                blk.set_instructions(out)
            else:
                blk.instructions = out


def _legalize_sync(nc):
    """Adapt the Tile-scheduled module to this container's walrus build:

    1. TPB instruction encodings here accept at most ONE inline sync wait
       ("Too many sync wait commands"), so hoist extra waits into standalone
       single-wait EventSemaphore instructions on the same engine.
    2. The tail EVENT_SEMAPHORE_RANGE_CLEAR InstISA is rejected ("ISA wrong
       length"); replace it with per-semaphore write-0 updates.
    """
    wid = [0]
    reset_done = set()   # sem ids already cleared once (drain + range-clear
                         # both cover the same range -- emit each id once)
    _eng_rr = [mybir.EngineType.Pool, mybir.EngineType.SP,
               mybir.EngineType.DVE, mybir.EngineType.Activation,
               mybir.EngineType.PE]

    def mk(engine, waits, updates):
        ev = mybir.InstEventSemaphore(name=f"WSPLIT-{wid[0]}")
        wid[0] += 1
        ev.engine = engine
        ev.sync_info = mybir.SyncInfo(on_wait=waits, on_update=updates)
        return ev

    for fn in nc.m.functions:
        for blk in fn.blocks:
            out = []
            for ins in blk.instructions:
                si = ins.sync_info
                if si is not None and len(si.on_wait) > 1:
                    for w in si.on_wait[:-1]:
                        out.append(mk(ins.engine, [w], []))
                    ins.sync_info = mybir.SyncInfo(
                        on_wait=[si.on_wait[-1]], on_update=list(si.on_update))
                if (type(ins).__name__ == "InstDrain"
                        and getattr(ins, "is_reset_sema", False)):
                    first = ins.reset_range_start
                    last = ins.reset_range_stop - 1
                    ins.is_reset_sema = False
                    ups = [mybir.SyncUpdate(sync_type="semaphore", id=s,
                                            update_mode="sem-wr-imm",
                                            update_value=0)
                           for s in range(first, last + 1)
                           if s not in reset_done]
                    reset_done.update(range(first, last + 1))
                    out.append(ins)
                    # all engines are quiescent behind the exit barrier at
                    # this point; spread the clears across them
                    for k, u in enumerate(ups):
                        out.append(mk(_eng_rr[k % len(_eng_rr)], [], [u]))
                    continue
                if (type(ins).__name__ == "InstISA"
                        and getattr(ins, "isa_opcode", None) == _RANGE_CLEAR_OPCODE):
                    import re as _re
                    m = _re.search(r"range_first=(\d+) range_last=(\d+)", str(ins))
                    first, last = int(m.group(1)), int(m.group(2))
                    ups = [mybir.SyncUpdate(sync_type="semaphore", id=s,
                                            update_mode="sem-wr-imm",
                                            update_value=0)
                           for s in range(first, last + 1)
                           if s not in reset_done]
                    reset_done.update(range(first, last + 1))
                    for u in ups:
                        out.append(mk(ins.engine, [], [u]))
                    continue
                out.append(ins)
            blk.set_instructions(out) if hasattr(blk, "set_instructions") else None
            if not hasattr(blk, "set_instructions"):
                blk.instructions = out


_NC_CACHE = {}


def build_nc(legalize=True, reps=1, loop=0):
    key = (legalize, reps, loop)
    if key in _NC_CACHE:
        return _NC_CACHE[key]
    nc = bass.Bass(enable_partition_id=(os.environ.get("KPID", "0") == "1"))
    io = {
        "wt": nc.declare_dram_parameter("wt", [128, FC * CP], _F8,
                                        isOutput=False),
        "xt": nc.declare_dram_parameter("xt", [128, FC * BLOC], _F8,
                                        isOutput=False),
        "accq": nc.declare_dram_parameter("accq", [128, BB * NROWS * C],
                                          _BF16, isOutput=True),
    }
    with tile.TileContext(nc) as tc_ctx:
        if loop:
            with tc_ctx.For_i(0, loop, 1):
                with ExitStack() as ctx:
                    _emit(ctx, tc_ctx, io)
        else:
            for _ in range(reps):
                with ExitStack() as ctx:
                    _emit(ctx, tc_ctx, io)
    if os.environ.get("KLDWDEDUP", "1") == "1":
        _dedup_ldweights(nc)
    if legalize:
        _legalize_sync(nc)
    _NC_CACHE[key] = nc
    return nc


def _colperm():
    """Device column order: col = s*SBW + c*T + t  <->  proto p = s*T + t."""
    s = np.arange(NSB)[:, None, None]
    c = np.arange(C)[None, :, None]
    t = np.arange(T)[None, None, :]
    return (c * P + s * T + t).reshape(-1)   # j index per device column


def make_in_maps(outputs, clusters, target_classes):
    X = np.asarray(outputs, dtype=np.float32)
    W = np.asarray(clusters, dtype=np.float32).reshape(CP, F)
    tcl = np.asarray(target_classes).astype(np.int64)

    w2b = (2.0 * W).astype(ml_dtypes.float8_e4m3)         # [CP, F]
    wf = w2b.astype(np.float32) * 0.5                     # W the device sees
    delta = (RECENTER - np.sum(wf * wf, axis=1))          # [CP]
    u = (delta / 64.0).astype(ml_dtypes.float8_e4m3)
    v = (delta - 64.0 * u.astype(np.float32)).astype(ml_dtypes.float8_e4m3)

    perm = _colperm()
    wcols = w2b.T[:, perm]                                # [F, CPdev]
    # wt host image matches the SBUF tile exactly: [128, NSB, FC, SBW]
    # (superblock-major so each superblock's bytes are contiguous/partition)
    wt = np.ascontiguousarray(
        wcols.reshape(FC, 128, NSB, SBW).transpose(1, 2, 0, 3))
    # bias rows replace features 510/511 (partitions 126/127 of fc=3)
    wt[126, :, 3, :] = u[perm].reshape(NSB, SBW)
    wt[127, :, 3, :] = v[perm].reshape(NSB, SBW)
    wt = wt.reshape(128, FC * CP)

    in_maps = []
    for cidx in range(N_CORES):
        xs = X[cidx * BLOC:(cidx + 1) * BLOC]             # [BLOC, F]
        xq = xs.astype(ml_dtypes.float8_e4m3)
        xt = np.ascontiguousarray(xq.T).reshape(FC, 128, BLOC)
        xt = np.ascontiguousarray(np.transpose(xt, (1, 0, 2)))  # [128,FC,BLOC]
        xt[126, 3, :] = np.float32(64.0)
        xt[127, 3, :] = np.float32(1.0)
        m = {"wt": wt, "xt": xt.reshape(128, FC * BLOC)}
        in_maps.append(m)
    return in_maps, X


def host_rows(results, target_classes):
    """Fold the shipped accumulators and select vt/vw per row (host side).

    Device accq[p, bb, slot, c] holds two partial per-class maxes of
    nq + RECENTER; row b = bb*128 + p of that core's shard.
    """
    tcl = np.asarray(target_classes).astype(np.int64)
    vt = np.empty(B, np.float64)
    vw = np.empty(B, np.float64)
    rows = np.arange(BLOC)
    for cidx, r in enumerate(results):
        a = r["accq"].astype(np.float32).reshape(128, BB, NROWS, C)
        maxq = a.max(axis=2)                       # [128, BB, C]
        maxq = maxq.transpose(1, 0, 2).reshape(BLOC, C).astype(np.float64)
        tc = tcl[cidx * BLOC:(cidx + 1) * BLOC]
        vt[cidx * BLOC:(cidx + 1) * BLOC] = maxq[rows, tc]
        m2 = maxq.copy()
        m2[rows, tc] = -np.inf
        vw[cidx * BLOC:(cidx + 1) * BLOC] = m2.max(axis=1)
    return vt - RECENTER, vw - RECENTER


def combine(results, X, target_classes):
    vt, vw = host_rows(results, target_classes)
    sx2 = float((X.astype(np.float64) ** 2).sum())
    tl = (sx2 - vt.sum()) / (B * F)
    ntl = (sx2 - vw.sum()) / (B * F)
    return np.float32((1.0 - ALPHA) * tl + ALPHA / (ntl + EPS))


def kernel(outputs, clusters, target_classes):
    nc = build_nc()
    in_maps, X = make_in_maps(outputs, clusters, target_classes)
    res = run_bass_kernel_spmd(nc, in_maps, core_ids=list(range(N_CORES))).results
    return combine(res, X, target_classes)



# revision 61
# speedup vs baseline: 1.1331x; 1.1331x over previous
"""Trainium2 Bass kernel: prototype-kNN CCE loss (nn_CCE_67190468378875).

Math: for each row b, the loss needs, per class, the min squared distance
over that class's 32 prototypes, evaluated at the target class (vt) and the
best non-target class (vw).  Equivalently per-proto score
nq[b,j] = 2 x_b.w_j - |w_j|^2; per-class MAX of nq gives -min d2 (+|x|^2).

Device work per core (batch-sharded 512 rows = 4 row-blocks of 128,
clusters replicated; prototype columns permuted plane-major; weights laid
out superblock-major in DRAM so every DMA chunk is contiguous/partition):

  psum[b, col] = 2 x_b . w_col + delta_col   (fp8 DoubleRow matmuls; the
      recentered bias delta = 512 - |w|^2 rides two sacrificial feature
      rows (64*u + v split), so no separate bias pass is needed).
      Superblock-PAIR schedule: per (pair q, row-block bb) 8 matmuls over
      two psum tiles; _dedup_ldweights() drops the per-matmul implicit
      LDWEIGHTS reloads (bass lowers every matmul into Ldweights+Matmult)
      down to 2 per pair.  Dependency-free warmup matmuls on a memset
      scratch bridge the DMA-gated head so the HAM clock gate (PE at
      1.2GHz until ~3.4us of sustained activity) releases before the real
      stream starts.
  consumers (seed-only, no accumulate chains or staging; host folds 20
      slot rows): even superblock of each pair: one ACT copies psum
      straight into 4 bf16 acc rows; odd superblock: one DVE TensorReduce
      over the 4 planes seeds one acc row (and frees the pair's lead psum
      tile fast).  Rows 0:15 ship after q=2, rows 15:20 after q=3
      (two-wave output DMA).
Host: input prep (fp8 cast, plane-major column permutation, feature-row
bias fold), 20-row max fold + vt/vw selection, final combine in f64.

This container's walrus build encodes at most ONE inline sync wait per TPB
instruction and rejects EVENT_SEMAPHORE_RANGE_CLEAR / INC_SWDGE_SEM ISA
ops and all gpsimd compute; _legalize_sync() post-processes the
Tile-scheduled module accordingly.
"""

import os
import numpy as np
import ml_dtypes
from contextlib import ExitStack

import concourse.bass as bass
import concourse.mybir as mybir
import concourse.tile as tile
from concourse.bass_utils import run_bass_kernel_spmd

B, C, P, F = 4096, 200, 32, 512
CP = C * P                  # 6400 prototypes
ALPHA, EPS = 5.0, 1e-8
N_CORES = 8
BLOC = B // N_CORES         # 512 rows per core
BB = BLOC // 128            # 4 row-blocks of 128
FC = F // 128               # 4 contraction chunks
T = 4                       # prototype planes per superblock
NSB = P // T                # 8 superblocks
SBW = C * T                 # 800 columns per superblock
PSUM_BUFS = int(os.environ.get("KPSB", "4"))  # psum pool tiles
PSW = int(os.environ.get("KPSW", "1024"))     # psum tile width (fp32 cols)
RECENTER = 512.0            # delta = RECENTER - |w|^2 rides the fold rows

_BF16 = mybir.dt.bfloat16
_F32 = mybir.dt.float32
_F8 = mybir.dt.float8e4

NROWS = 20                  # slot rows per row-block shipped to the host


def _emit(ctx, tc_ctx, io):
    """Superblock-pair schedule; see module docstring."""
    nc = tc_ctx.nc
    singles = ctx.enter_context(tc_ctx.tile_pool(name="singles", bufs=1))
    psum = ctx.enter_context(tc_ctx.tile_pool(name="psum", bufs=PSUM_BUFS,
                                              space="PSUM"))

    # weights laid out superblock-major so every DMA chunk is contiguous
    # per partition (one fat descriptor instead of FC strided stripes)
    wt_t = singles.tile([128, NSB, FC, SBW], _F8)
    xt_t = singles.tile([128, FC, BLOC], _F8)    # X^T (+ fold const rows)
    acc = singles.tile([128, BB, NROWS, C], _BF16)  # per-row-block slot rows
    wrm = singles.tile([128, 2, 128], _F8)       # HAM warmup scratch

    pm = mybir.MatmulPerfMode.DoubleRow

    # Input DMA dispatches cost ~0.65us of dispatcher-engine time each; put
    # wt superblock 0 first (it gates the first real matmul) and xt on the
    # scalar engine so the two leading feeds start in parallel.
    wt_in = io["wt"][:, :].rearrange("p (s fc j) -> p s fc j", s=NSB, fc=FC)
    nc.sync.dma_start(out=wt_t[:, 0:1], in_=wt_in[:, 0:1])
    nc.scalar.dma_start(
        out=xt_t[:, :, :],
        in_=io["xt"][:, :].rearrange("p (fc b) -> p fc b", fc=FC))
    for s0, s1 in ((1, 2), (2, 3), (3, 4), (4, 6), (6, 8)):
        nc.sync.dma_start(out=wt_t[:, s0:s1], in_=wt_in[:, s0:s1])

    # HAM warmup: the PE clock-gate sits at 1.2GHz until ~3.4us of sustained
    # matmul activity.  Spin dependency-free garbage matmuls during the DMA
    # head so the real stream runs at 2.4GHz from its first instruction.
    n_warm = int(os.environ.get("KWARM", "36"))
    if n_warm:
        nc.vector.memset(wrm[:, :, :], 0.0)
        pw = psum.tile([128, PSW], _F32, tag="ps")
        for _ in range(n_warm):
            nc.tensor.matmul(pw[:, 0:128], wrm[:, :, 0:128], wrm[:, :, :],
                             start=True, stop=True, perf_mode=pm,
                             skip_group_check=True)

    NQ = NSB // 2            # 4 superblock pairs
    order = [(q, bb) for q in range(NQ - 1) for bb in range(BB)]
    order += [(NQ - 1, bb) for bb in range(BB - 1, -1, -1)]

    # seed-only consumer plan: every drain seeds its own slot rows in acc
    # (no accumulate chains, no staging, no TT merges at all), host folds
    # the NROWS rows per row-block.  Row layout is wave-ordered so rows
    # 0:15 are final after the q=2 pair (shipped mid-stream) and only rows
    # 15:20 ride the tail:
    #   h0 (even sb):  ACT copies psum straight into 4 acc rows
    #                  (q0->0:4, q1->4:8, q2->8:12, q3->15:19)
    #   h1 (odd sb):   TR over the 4 planes seeds one acc row
    #                  (q0->12, q1->13, q2->14, q3->19)
    def consume0(q, ps, bb):     # h0 chain: ACT seeds 4 rows directly
        r0 = {0: 0, 1: 4, 2: 8, 3: 15}[q]
        psv = ps[:, 0:SBW].rearrange("p (c t) -> p t c", t=T)
        nc.scalar.activation(acc[:, bb, r0:r0 + T, :], psv,
                             mybir.ActivationFunctionType.Copy)

    def consume1(q, ps, bb):     # h1 chain: direct DVE reduce, frees psum
        row = {0: 12, 1: 13, 2: 14, 3: 19}[q]
        rin = ps[:, 0:SBW].rearrange("p (c t) -> p c t", t=T)
        nc.vector.tensor_reduce(
            out=acc[:, bb, row, :], in_=rin,
            axis=mybir.AxisListType.X, op=mybir.AluOpType.max)

    for q, bb in order:
        s0, s1 = 2 * q, 2 * q + 1
        ps0 = psum.tile([128, PSW], _F32, tag="ps")
        ps1 = psum.tile([128, PSW], _F32, tag="ps")
        rows = slice(bb * 128, (bb + 1) * 128)
        for pi in range(2):
            for s, ps in ((s0, ps0), (s1, ps1)):
                for p0, p1 in ((0, 512), (512, SBW)):
                    nc.tensor.matmul(
                        ps[:, p0:p1],
                        xt_t[:, 2 * pi:2 * pi + 2, rows],
                        wt_t[:, s, 2 * pi:2 * pi + 2, p0:p1],
                        start=(pi == 0), stop=(pi == 1), perf_mode=pm)

        consume1(q, ps0, bb)   # TR drains the pair's FIRST psum tile: the
        consume0(q, ps1, bb)   # next-next pair's lead MMs see a fast free

        if q == NQ - 2:
            nc.sync.dma_start(
                out=io["accq"][:, bb * NROWS * C:bb * NROWS * C + 15 * C],
                in_=acc[:, bb, 0:15, :].rearrange("p r c -> p (r c)"))
        elif q == NQ - 1:
            nc.sync.dma_start(
                out=io["accq"][:, bb * NROWS * C + 15 * C:
                               (bb + 1) * NROWS * C],
                in_=acc[:, bb, 15:20, :].rearrange("p r c -> p (r c)"))


_RANGE_CLEAR_OPCODE = 176


def _dedup_ldweights(nc):
    """Drop InstLdweights whose stationary AP matches the previous Ldweights
    on the PE stream (bass lowers every matmul into Ldweights + Matmult;
    consecutive matmuls sharing lhsT reload identical weights for nothing --
    each reload is ~126ns of PE time plus sequencer pressure).

    A dropped Ldweights' waits and sem updates migrate to the next kept PE
    instruction so cross-engine vector clocks stay intact.
    """
    for fn in nc.m.functions:
        for blk in fn.blocks:
            last_key = None
            pend_waits, pend_ups = [], []
            out = []
            for ins in blk.instructions:
                tn = type(ins).__name__
                if getattr(ins, "engine", None) != mybir.EngineType.PE:
                    out.append(ins)
                    continue
                if tn == "InstLdweights":
                    key = (str(ins.ins[0]), str(ins.perf_mode),
                           str(getattr(ins, "tile_position", None)))
                    si = ins.sync_info
                    if key == last_key:
                        if si is not None:
                            pend_waits += list(si.on_wait)
                            pend_ups += list(si.on_update)
                        continue
                    last_key = key
                if pend_waits or pend_ups:
                    si = ins.sync_info
                    waits = list(si.on_wait) if si else []
                    ups = list(si.on_update) if si else []
                    # merge duplicate sem-inc updates by summing values
                    for u in pend_ups:
                        for v in ups:
                            if (u.sync_type == v.sync_type
                                    and getattr(u, "id", None) == getattr(v, "id", None)
                                    and u.update_mode == v.update_mode == "sem-inc"):
                                v.update_value += u.update_value
                                break
                        else:
                            ups.append(u)
                    ins.sync_info = mybir.SyncInfo(
                        on_wait=pend_waits + waits, on_update=ups)
                    pend_waits, pend_ups = [], []
                out.append(ins)
            assert not pend_waits and not pend_ups, (
                "dangling sync from dropped trailing Ldweights")
            if hasattr(blk, "set_instructions"):
                blk.set_instructions(out)
            else:
                blk.instructions = out


def _legalize_sync(nc):
    """Adapt the Tile-scheduled module to this container's walrus build:

    1. TPB instruction encodings here accept at most ONE inline sync wait
       ("Too many sync wait commands"), so hoist extra waits into standalone
       single-wait EventSemaphore instructions on the same engine.
    2. The tail EVENT_SEMAPHORE_RANGE_CLEAR InstISA is rejected ("ISA wrong
       length"); replace it with per-semaphore write-0 updates.
    """
    wid = [0]
    reset_done = set()   # sem ids already cleared once (drain + range-clear
                         # both cover the same range -- emit each id once)
    _eng_rr = [mybir.EngineType.Pool, mybir.EngineType.SP,
               mybir.EngineType.DVE, mybir.EngineType.Activation,
               mybir.EngineType.PE]

    def mk(engine, waits, updates):
        ev = mybir.InstEventSemaphore(name=f"WSPLIT-{wid[0]}")
        wid[0] += 1
        ev.engine = engine
        ev.sync_info = mybir.SyncInfo(on_wait=waits, on_update=updates)
        return ev

    for fn in nc.m.functions:
        for blk in fn.blocks:
            out = []
            for ins in blk.instructions:
                si = ins.sync_info
                if si is not None and len(si.on_wait) > 1:
                    for w in si.on_wait[:-1]:
                        out.append(mk(ins.engine, [w], []))
                    ins.sync_info = mybir.SyncInfo(
                        on_wait=[si.on_wait[-1]], on_update=list(si.on_update))
                if (type(ins).__name__ == "InstDrain"
                        and getattr(ins, "is_reset_sema", False)):
                    first = ins.reset_range_start
                    last = ins.reset_range_stop - 1
                    ins.is_reset_sema = False
                    ups = [mybir.SyncUpdate(sync_type="semaphore", id=s,
                                            update_mode="sem-wr-imm",
                                            update_value=0)
                           for s in range(first, last + 1)
                           if s not in reset_done]
                    reset_done.update(range(first, last + 1))
                    out.append(ins)
                    # all engines are quiescent behind the exit barrier at
                    # this point; spread the clears across them
                    for k, u in enumerate(ups):
                        out.append(mk(_eng_rr[k % len(_eng_rr)], [], [u]))
                    continue
                if (type(ins).__name__ == "InstISA"
                        and getattr(ins, "isa_opcode", None) == _RANGE_CLEAR_OPCODE):
                    import re as _re
                    m = _re.search(r"range_first=(\d+) range_last=(\d+)", str(ins))
                    first, last = int(m.group(1)), int(m.group(2))
                    ups = [mybir.SyncUpdate(sync_type="semaphore", id=s,
                                            update_mode="sem-wr-imm",
                                            update_value=0)
                           for s in range(first, last + 1)
                           if s not in reset_done]
                    reset_done.update(range(first, last + 1))
                    for u in ups:
                        out.append(mk(ins.engine, [], [u]))
                    continue
                out.append(ins)
            blk.set_instructions(out) if hasattr(blk, "set_instructions") else None
            if not hasattr(blk, "set_instructions"):
                blk.instructions = out


_NC_CACHE = {}


def build_nc(legalize=True, reps=1, loop=0):
    key = (legalize, reps, loop)
    if key in _NC_CACHE:
        return _NC_CACHE[key]
    nc = bass.Bass(enable_partition_id=(os.environ.get("KPID", "0") == "1"))
    io = {
        "wt": nc.declare_dram_parameter("wt", [128, FC * CP], _F8,
                                        isOutput=False),
        "xt": nc.declare_dram_parameter("xt", [128, FC * BLOC], _F8,
                                        isOutput=False),
        "accq": nc.declare_dram_parameter("accq", [128, BB * NROWS * C],
                                          _BF16, isOutput=True),
    }
    with tile.TileContext(nc) as tc_ctx:
        if loop:
            with tc_ctx.For_i(0, loop, 1):
                with ExitStack() as ctx:
                    _emit(ctx, tc_ctx, io)
        else:
            for _ in range(reps):
                with ExitStack() as ctx:
                    _emit(ctx, tc_ctx, io)
    if os.environ.get("KLDWDEDUP", "1") == "1":
        _dedup_ldweights(nc)
    if legalize:
        _legalize_sync(nc)
    _NC_CACHE[key] = nc
    return nc


def _colperm():
    """Device column order: col = s*SBW + c*T + t  <->  proto p = s*T + t."""
    s = np.arange(NSB)[:, None, None]
    c = np.arange(C)[None, :, None]
    t = np.arange(T)[None, None, :]
    return (c * P + s * T + t).reshape(-1)   # j index per device column


def make_in_maps(outputs, clusters, target_classes):
    X = np.asarray(outputs, dtype=np.float32)
    W = np.asarray(clusters, dtype=np.float32).reshape(CP, F)
    tcl = np.asarray(target_classes).astype(np.int64)

    w2b = (2.0 * W).astype(ml_dtypes.float8_e4m3)         # [CP, F]
    wf = w2b.astype(np.float32) * 0.5                     # W the device sees
    delta = (RECENTER - np.sum(wf * wf, axis=1))          # [CP]
    u = (delta / 64.0).astype(ml_dtypes.float8_e4m3)
    v = (delta - 64.0 * u.astype(np.float32)).astype(ml_dtypes.float8_e4m3)

    perm = _colperm()
    wcols = w2b.T[:, perm]                                # [F, CPdev]
    # wt host image matches the SBUF tile exactly: [128, NSB, FC, SBW]
    # (superblock-major so each superblock's bytes are contiguous/partition)
    wt = np.ascontiguousarray(
        wcols.reshape(FC, 128, NSB, SBW).transpose(1, 2, 0, 3))
    # bias rows replace features 510/511 (partitions 126/127 of fc=3)
    wt[126, :, 3, :] = u[perm].reshape(NSB, SBW)
    wt[127, :, 3, :] = v[perm].reshape(NSB, SBW)
    wt = wt.reshape(128, FC * CP)

    in_maps = []
    for cidx in range(N_CORES):
        xs = X[cidx * BLOC:(cidx + 1) * BLOC]             # [BLOC, F]
        xq = xs.astype(ml_dtypes.float8_e4m3)
        xt = np.ascontiguousarray(xq.T).reshape(FC, 128, BLOC)
        xt = np.ascontiguousarray(np.transpose(xt, (1, 0, 2)))  # [128,FC,BLOC]
        xt[126, 3, :] = np.float32(64.0)
        xt[127, 3, :] = np.float32(1.0)
        m = {"wt": wt, "xt": xt.reshape(128, FC * BLOC)}
        in_maps.append(m)
    return in_maps, X


def host_rows(results, target_classes):
    """Fold the shipped accumulators and select vt/vw per row (host side).

    Device accq[p, bb, slot, c] holds two partial per-class maxes of
    nq + RECENTER; row b = bb*128 + p of that core's shard.
    """
    tcl = np.asarray(target_classes).astype(np.int64)
    vt = np.empty(B, np.float64)
    vw = np.empty(B, np.float64)
    rows = np.arange(BLOC)
    for cidx, r in enumerate(results):
        a = r["accq"].astype(np.float32).reshape(128, BB, NROWS, C)
        maxq = a.max(axis=2)                       # [128, BB, C]
        maxq = maxq.transpose(1, 0, 2).reshape(BLOC, C).astype(np.float64)
        tc = tcl[cidx * BLOC:(cidx + 1) * BLOC]
        vt[cidx * BLOC:(cidx + 1) * BLOC] = maxq[rows, tc]
        m2 = maxq.copy()
        m2[rows, tc] = -np.inf
        vw[cidx * BLOC:(cidx + 1) * BLOC] = m2.max(axis=1)
    return vt - RECENTER, vw - RECENTER


def combine(results, X, target_classes):
    vt, vw = host_rows(results, target_classes)
    sx2 = float((X.astype(np.float64) ** 2).sum())
    tl = (sx2 - vt.sum()) / (B * F)
    ntl = (sx2 - vw.sum()) / (B * F)
    return np.float32((1.0 - ALPHA) * tl + ALPHA / (ntl + EPS))


def kernel(outputs, clusters, target_classes):
    nc = build_nc()
    in_maps, X = make_in_maps(outputs, clusters, target_classes)
    res = run_bass_kernel_spmd(nc, in_maps, core_ids=list(range(N_CORES))).results
    return combine(res, X, target_classes)



# revision 63
# speedup vs baseline: 1.1415x; 1.0074x over previous
"""Trainium2 Bass kernel: prototype-kNN CCE loss (nn_CCE_67190468378875).

Math: for each row b, the loss needs, per class, the min squared distance
over that class's 32 prototypes, evaluated at the target class (vt) and the
best non-target class (vw).  Equivalently per-proto score
nq[b,j] = 2 x_b.w_j - |w_j|^2; per-class MAX of nq gives -min d2 (+|x|^2).

Device work per core (batch-sharded 512 rows = 4 row-blocks of 128,
clusters replicated; prototype columns permuted plane-major; weights laid
out superblock-major in DRAM so every DMA chunk is contiguous/partition):

  psum[b, col] = 2 x_b . w_col + delta_col   (fp8 DoubleRow matmuls; the
      recentered bias delta = 512 - |w|^2 rides two sacrificial feature
      rows (64*u + v split), so no separate bias pass is needed).
      Superblock-PAIR schedule: per (pair q, row-block bb) 8 matmuls over
      two psum tiles; _dedup_ldweights() drops the per-matmul implicit
      LDWEIGHTS reloads (bass lowers every matmul into Ldweights+Matmult)
      down to 2 per pair.  Dependency-free warmup matmuls on a memset
      scratch bridge the DMA-gated head so the HAM clock gate (PE at
      1.2GHz until ~3.4us of sustained activity) releases before the real
      stream starts.
  consumers (seed-only, no accumulate chains or staging; host folds 20
      slot rows): even superblock of each pair: one ACT copies psum
      straight into 4 bf16 acc rows; odd superblock: one DVE TensorReduce
      over the 4 planes seeds one acc row (and frees the pair's lead psum
      tile fast).  Rows 0:15 ship after q=2, rows 15:20 after q=3
      (two-wave output DMA).
Host: input prep (fp8 cast, plane-major column permutation, feature-row
bias fold), 20-row max fold + vt/vw selection, final combine in f64.

This container's walrus build encodes at most ONE inline sync wait per TPB
instruction and rejects EVENT_SEMAPHORE_RANGE_CLEAR / INC_SWDGE_SEM ISA
ops and all gpsimd compute; _legalize_sync() post-processes the
Tile-scheduled module accordingly.
"""

import os
import numpy as np
import ml_dtypes
from contextlib import ExitStack

import concourse.bass as bass
import concourse.mybir as mybir
import concourse.tile as tile
from concourse.bass_utils import run_bass_kernel_spmd

B, C, P, F = 4096, 200, 32, 512
CP = C * P                  # 6400 prototypes
ALPHA, EPS = 5.0, 1e-8
N_CORES = 8
BLOC = B // N_CORES         # 512 rows per core
BB = BLOC // 128            # 4 row-blocks of 128
FC = F // 128               # 4 contraction chunks
T = 4                       # prototype planes per superblock
NSB = P // T                # 8 superblocks
SBW = C * T                 # 800 columns per superblock
PSUM_BUFS = int(os.environ.get("KPSB", "4"))  # psum pool tiles
PSW = int(os.environ.get("KPSW", "1024"))     # psum tile width (fp32 cols)
RECENTER = 512.0            # delta = RECENTER - |w|^2 rides the fold rows

_BF16 = mybir.dt.bfloat16
_F32 = mybir.dt.float32
_F8 = mybir.dt.float8e4

NROWS = 20                  # slot rows per row-block shipped to the host


def _emit(ctx, tc_ctx, io):
    """Superblock-pair schedule; see module docstring."""
    nc = tc_ctx.nc
    singles = ctx.enter_context(tc_ctx.tile_pool(name="singles", bufs=1))
    psum = ctx.enter_context(tc_ctx.tile_pool(name="psum", bufs=PSUM_BUFS,
                                              space="PSUM"))

    # weights laid out superblock-major so every DMA chunk is contiguous
    # per partition (one fat descriptor instead of FC strided stripes)
    wt_t = singles.tile([128, NSB, FC, SBW], _F8)
    xt_t = singles.tile([128, FC, BLOC], _F8)    # X^T (+ fold const rows)
    acc = singles.tile([128, BB, NROWS, C], _BF16)  # per-row-block slot rows
    wrm = singles.tile([128, 2, 128], _F8)       # HAM warmup scratch

    pm = mybir.MatmulPerfMode.DoubleRow

    # Input DMA dispatches cost ~0.65us of dispatcher-engine time each; put
    # wt superblock 0 first (it gates the first real matmul) and xt on the
    # scalar engine so the two leading feeds start in parallel.
    wt_in = io["wt"][:, :].rearrange("p (s fc j) -> p s fc j", s=NSB, fc=FC)
    nc.sync.dma_start(out=wt_t[:, 0:1], in_=wt_in[:, 0:1])
    nc.scalar.dma_start(
        out=xt_t[:, :, :],
        in_=io["xt"][:, :].rearrange("p (fc b) -> p fc b", fc=FC))
    for s0, s1 in ((1, 2), (2, 3), (3, 4), (4, 6), (6, 8)):
        nc.sync.dma_start(out=wt_t[:, s0:s1], in_=wt_in[:, s0:s1])

    # HAM warmup: the PE clock-gate sits at 1.2GHz until ~3.4us of sustained
    # matmul activity.  Spin dependency-free garbage matmuls during the DMA
    # head so the real stream runs at 2.4GHz from its first instruction.
    n_warm = int(os.environ.get("KWARM", "36"))
    if n_warm:
        nc.vector.memset(wrm[:, :, :], 0.0)
        pw = psum.tile([128, PSW], _F32, tag="ps")
        for _ in range(n_warm):
            nc.tensor.matmul(pw[:, 0:128], wrm[:, :, 0:128], wrm[:, :, :],
                             start=True, stop=True, perf_mode=pm,
                             skip_group_check=True)

    NQ = NSB // 2            # 4 superblock pairs
    order = [(q, bb) for q in range(NQ - 1) for bb in range(BB)]
    order += [(NQ - 1, bb) for bb in range(BB - 1, -1, -1)]

    # seed-only consumer plan: every drain seeds its own slot rows in acc
    # (no accumulate chains, no staging, no TT merges at all), host folds
    # the NROWS rows per row-block.  Row layout is wave-ordered so rows
    # 0:15 are final after the q=2 pair (shipped mid-stream) and only rows
    # 15:20 ride the tail:
    #   h0 (even sb):  ACT copies psum straight into 4 acc rows
    #                  (q0->0:4, q1->4:8, q2->8:12, q3->15:19)
    #   h1 (odd sb):   TR over the 4 planes seeds one acc row
    #                  (q0->12, q1->13, q2->14, q3->19)
    def consume0(q, ps, bb):     # h0 chain: ACT seeds 4 rows directly
        r0 = {0: 0, 1: 4, 2: 8, 3: 15}[q]
        psv = ps[:, 0:SBW].rearrange("p (c t) -> p t c", t=T)
        nc.scalar.activation(acc[:, bb, r0:r0 + T, :], psv,
                             mybir.ActivationFunctionType.Copy)

    def consume1(q, ps, bb):     # h1 chain: direct DVE reduce, frees psum
        row = {0: 12, 1: 13, 2: 14, 3: 19}[q]
        rin = ps[:, 0:SBW].rearrange("p (c t) -> p c t", t=T)
        nc.vector.tensor_reduce(
            out=acc[:, bb, row, :], in_=rin,
            axis=mybir.AxisListType.X, op=mybir.AluOpType.max)

    for q, bb in order:
        s0, s1 = 2 * q, 2 * q + 1
        ps0 = psum.tile([128, PSW], _F32, tag="ps")
        ps1 = psum.tile([128, PSW], _F32, tag="ps")
        rows = slice(bb * 128, (bb + 1) * 128)
        for pi in range(2):
            for s, ps in ((s0, ps0), (s1, ps1)):
                for p0, p1 in ((0, 512), (512, SBW)):
                    nc.tensor.matmul(
                        ps[:, p0:p1],
                        xt_t[:, 2 * pi:2 * pi + 2, rows],
                        wt_t[:, s, 2 * pi:2 * pi + 2, p0:p1],
                        start=(pi == 0), stop=(pi == 1), perf_mode=pm)

        consume1(q, ps0, bb)   # TR drains the pair's FIRST psum tile: the
        consume0(q, ps1, bb)   # next-next pair's lead MMs see a fast free

        if q == NQ - 2:
            nc.sync.dma_start(
                out=io["accq"][:, bb * NROWS * C:bb * NROWS * C + 15 * C],
                in_=acc[:, bb, 0:15, :].rearrange("p r c -> p (r c)"))
        elif q == NQ - 1:
            nc.sync.dma_start(
                out=io["accq"][:, bb * NROWS * C + 15 * C:
                               (bb + 1) * NROWS * C],
                in_=acc[:, bb, 15:20, :].rearrange("p r c -> p (r c)"))


_RANGE_CLEAR_OPCODE = 176


def _dedup_ldweights(nc):
    """Drop InstLdweights whose stationary AP matches the previous Ldweights
    on the PE stream (bass lowers every matmul into Ldweights + Matmult;
    consecutive matmuls sharing lhsT reload identical weights for nothing --
    each reload is ~126ns of PE time plus sequencer pressure).

    A dropped Ldweights' waits and sem updates migrate to the next kept PE
    instruction so cross-engine vector clocks stay intact.
    """
    for fn in nc.m.functions:
        for blk in fn.blocks:
            last_key = None
            pend_waits, pend_ups = [], []
            out = []
            for ins in blk.instructions:
                tn = type(ins).__name__
                if getattr(ins, "engine", None) != mybir.EngineType.PE:
                    out.append(ins)
                    continue
                if tn == "InstLdweights":
                    key = (str(ins.ins[0]), str(ins.perf_mode),
                           str(getattr(ins, "tile_position", None)))
                    si = ins.sync_info
                    if key == last_key:
                        if si is not None:
                            pend_waits += list(si.on_wait)
                            pend_ups += list(si.on_update)
                        continue
                    last_key = key
                if pend_waits or pend_ups:
                    si = ins.sync_info
                    waits = list(si.on_wait) if si else []
                    ups = list(si.on_update) if si else []
                    # merge duplicate sem-inc updates by summing values
                    for u in pend_ups:
                        for v in ups:
                            if (u.sync_type == v.sync_type
                                    and getattr(u, "id", None) == getattr(v, "id", None)
                                    and u.update_mode == v.update_mode == "sem-inc"):
                                v.update_value += u.update_value
                                break
                        else:
                            ups.append(u)
                    ins.sync_info = mybir.SyncInfo(
                        on_wait=pend_waits + waits, on_update=ups)
                    pend_waits, pend_ups = [], []
                out.append(ins)
            assert not pend_waits and not pend_ups, (
                "dangling sync from dropped trailing Ldweights")
            if hasattr(blk, "set_instructions"):
                blk.set_instructions(out)
            else:
                blk.instructions = out


def _legalize_sync(nc):
    """Adapt the Tile-scheduled module to this container's walrus build:

    1. TPB instruction encodings here accept at most ONE inline sync wait
       ("Too many sync wait commands"), so hoist extra waits into standalone
       single-wait EventSemaphore instructions on the same engine.
    2. The tail EVENT_SEMAPHORE_RANGE_CLEAR InstISA is rejected ("ISA wrong
       length"); replace it with per-semaphore write-0 updates.
    """
    wid = [0]
    reset_done = set()   # sem ids already cleared once (drain + range-clear
                         # both cover the same range -- emit each id once)
    _eng_rr = [mybir.EngineType.Pool, mybir.EngineType.SP,
               mybir.EngineType.DVE, mybir.EngineType.Activation,
               mybir.EngineType.PE]

    def mk(engine, waits, updates):
        ev = mybir.InstEventSemaphore(name=f"WSPLIT-{wid[0]}")
        wid[0] += 1
        ev.engine = engine
        ev.sync_info = mybir.SyncInfo(on_wait=waits, on_update=updates)
        return ev

    for fn in nc.m.functions:
        for blk in fn.blocks:
            out = []
            for ins in blk.instructions:
                si = ins.sync_info
                if si is not None and len(si.on_wait) > 1:
                    for w in si.on_wait[:-1]:
                        out.append(mk(ins.engine, [w], []))
                    ins.sync_info = mybir.SyncInfo(
                        on_wait=[si.on_wait[-1]], on_update=list(si.on_update))
                if (type(ins).__name__ == "InstDrain"
                        and getattr(ins, "is_reset_sema", False)):
                    first = ins.reset_range_start
                    last = ins.reset_range_stop - 1
                    ins.is_reset_sema = False
                    ups = [mybir.SyncUpdate(sync_type="semaphore", id=s,
                                            update_mode="sem-wr-imm",
                                            update_value=0)
                           for s in range(first, last + 1)
                           if s not in reset_done]
                    reset_done.update(range(first, last + 1))
                    out.append(ins)
                    # all engines are quiescent behind the exit barrier at
                    # this point; spread the clears across them
                    for k, u in enumerate(ups):
                        out.append(mk(_eng_rr[k % len(_eng_rr)], [], [u]))
                    continue
                if (type(ins).__name__ == "InstISA"
                        and getattr(ins, "isa_opcode", None) == _RANGE_CLEAR_OPCODE):
                    import re as _re
                    m = _re.search(r"range_first=(\d+) range_last=(\d+)", str(ins))
                    first, last = int(m.group(1)), int(m.group(2))
                    ups = [mybir.SyncUpdate(sync_type="semaphore", id=s,
                                            update_mode="sem-wr-imm",
                                            update_value=0)
                           for s in range(first, last + 1)
                           if s not in reset_done]
                    reset_done.update(range(first, last + 1))
                    for u in ups:
                        out.append(mk(ins.engine, [], [u]))
                    continue
                out.append(ins)
            blk.set_instructions(out) if hasattr(blk, "set_instructions") else None
            if not hasattr(blk, "set_instructions"):
                blk.instructions = out


_NC_CACHE = {}


def build_nc(legalize=True, reps=1, loop=0):
    key = (legalize, reps, loop)
    if key in _NC_CACHE:
        return _NC_CACHE[key]
    nc = bass.Bass(enable_partition_id=(os.environ.get("KPID", "0") == "1"))
    io = {
        "wt": nc.declare_dram_parameter("wt", [128, FC * CP], _F8,
                                        isOutput=False),
        "xt": nc.declare_dram_parameter("xt", [128, FC * BLOC], _F8,
                                        isOutput=False),
        "accq": nc.declare_dram_parameter("accq", [128, BB * NROWS * C],
                                          _BF16, isOutput=True),
    }
    with tile.TileContext(nc) as tc_ctx:
        if loop:
            with tc_ctx.For_i(0, loop, 1):
                with ExitStack() as ctx:
                    _emit(ctx, tc_ctx, io)
        else:
            for _ in range(reps):
                with ExitStack() as ctx:
                    _emit(ctx, tc_ctx, io)
    if os.environ.get("KLDWDEDUP", "1") == "1":
        _dedup_ldweights(nc)
    if legalize:
        _legalize_sync(nc)
    _NC_CACHE[key] = nc
    return nc


def _colperm():
    """Device column order: col = s*SBW + c*T + t  <->  proto p = s*T + t."""
    s = np.arange(NSB)[:, None, None]
    c = np.arange(C)[None, :, None]
    t = np.arange(T)[None, None, :]
    return (c * P + s * T + t).reshape(-1)   # j index per device column


def make_in_maps(outputs, clusters, target_classes):
    X = np.asarray(outputs, dtype=np.float32)
    W = np.asarray(clusters, dtype=np.float32).reshape(CP, F)
    tcl = np.asarray(target_classes).astype(np.int64)

    w2b = (2.0 * W).astype(ml_dtypes.float8_e4m3)         # [CP, F]
    wf = w2b.astype(np.float32) * 0.5                     # W the device sees
    delta = (RECENTER - np.sum(wf * wf, axis=1))          # [CP]
    u = (delta / 64.0).astype(ml_dtypes.float8_e4m3)
    v = (delta - 64.0 * u.astype(np.float32)).astype(ml_dtypes.float8_e4m3)

    perm = _colperm()
    wcols = w2b.T[:, perm]                                # [F, CPdev]
    # wt host image matches the SBUF tile exactly: [128, NSB, FC, SBW]
    # (superblock-major so each superblock's bytes are contiguous/partition)
    wt = np.ascontiguousarray(
        wcols.reshape(FC, 128, NSB, SBW).transpose(1, 2, 0, 3))
    # bias rows replace features 510/511 (partitions 126/127 of fc=3)
    wt[126, :, 3, :] = u[perm].reshape(NSB, SBW)
    wt[127, :, 3, :] = v[perm].reshape(NSB, SBW)
    wt = wt.reshape(128, FC * CP)

    in_maps = []
    for cidx in range(N_CORES):
        xs = X[cidx * BLOC:(cidx + 1) * BLOC]             # [BLOC, F]
        xq = xs.astype(ml_dtypes.float8_e4m3)
        xt = np.ascontiguousarray(xq.T).reshape(FC, 128, BLOC)
        xt = np.ascontiguousarray(np.transpose(xt, (1, 0, 2)))  # [128,FC,BLOC]
        xt[126, 3, :] = np.float32(64.0)
        xt[127, 3, :] = np.float32(1.0)
        m = {"wt": wt, "xt": xt.reshape(128, FC * BLOC)}
        in_maps.append(m)
    return in_maps, X


def host_rows(results, target_classes):
    """Fold the shipped accumulators and select vt/vw per row (host side).

    Device accq[p, bb, slot, c] holds two partial per-class maxes of
    nq + RECENTER; row b = bb*128 + p of that core's shard.
    """
    tcl = np.asarray(target_classes).astype(np.int64)
    vt = np.empty(B, np.float64)
    vw = np.empty(B, np.float64)
    rows = np.arange(BLOC)
    for cidx, r in enumerate(results):
        a = r["accq"].astype(np.float32).reshape(128, BB, NROWS, C)
        maxq = a.max(axis=2)                       # [128, BB, C]
        maxq = maxq.transpose(1, 0, 2).reshape(BLOC, C).astype(np.float64)
        tc = tcl[cidx * BLOC:(cidx + 1) * BLOC]
        vt[cidx * BLOC:(cidx + 1) * BLOC] = maxq[rows, tc]
        m2 = maxq.copy()
        m2[rows, tc] = -np.inf
        vw[cidx * BLOC:(cidx + 1) * BLOC] = m2.max(axis=1)
    return vt - RECENTER, vw - RECENTER


def combine(results, X, target_classes):
    vt, vw = host_rows(results, target_classes)
    sx2 = float((X.astype(np.float64) ** 2).sum())
    tl = (sx2 - vt.sum()) / (B * F)
    ntl = (sx2 - vw.sum()) / (B * F)
    return np.float32((1.0 - ALPHA) * tl + ALPHA / (ntl + EPS))


def kernel(outputs, clusters, target_classes):
    nc = build_nc()
    in_maps, X = make_in_maps(outputs, clusters, target_classes)
    res = run_bass_kernel_spmd(nc, in_maps, core_ids=list(range(N_CORES))).results
    return combine(res, X, target_classes)



# revision 65
# speedup vs baseline: 1.1884x; 1.0411x over previous
"""Trainium2 Bass kernel: prototype-kNN CCE loss (nn_CCE_67190468378875).

Math: for each row b, the loss needs, per class, the min squared distance
over that class's 32 prototypes, evaluated at the target class (vt) and the
best non-target class (vw).  Equivalently per-proto score
nq[b,j] = 2 x_b.w_j - |w_j|^2; per-class MAX of nq gives -min d2 (+|x|^2).

Device work per core (batch-sharded 512 rows = 4 row-blocks of 128,
clusters replicated; prototype columns permuted plane-major; weights laid
out superblock-major in DRAM so every DMA chunk is contiguous/partition):

  psum[b, col] = 2 x_b . w_col + delta_col   (fp8 DoubleRow matmuls; the
      recentered bias delta = 512 - |w|^2 rides two sacrificial feature
      rows (64*u + v split), so no separate bias pass is needed).
      Superblock-PAIR schedule: per (pair q, row-block bb) 8 matmuls over
      two psum tiles; _dedup_ldweights() drops the per-matmul implicit
      LDWEIGHTS reloads (bass lowers every matmul into Ldweights+Matmult)
      down to 2 per pair.  Dependency-free warmup matmuls on a memset
      scratch bridge the DMA-gated head so the HAM clock gate (PE at
      1.2GHz until ~3.4us of sustained activity) releases before the real
      stream starts.
  consumers (seed-only, no accumulate chains or staging; host folds 20
      slot rows): even superblock of each pair: one ACT copies psum
      straight into 4 bf16 acc rows; odd superblock: one DVE TensorReduce
      over the 4 planes seeds one acc row (and frees the pair's lead psum
      tile fast).  Rows 0:15 ship after q=2, rows 15:20 after q=3
      (two-wave output DMA).
Host: input prep (fp8 cast, plane-major column permutation, feature-row
bias fold), 20-row max fold + vt/vw selection, final combine in f64.

This container's walrus build encodes at most ONE inline sync wait per TPB
instruction and rejects EVENT_SEMAPHORE_RANGE_CLEAR / INC_SWDGE_SEM ISA
ops and all gpsimd compute; _legalize_sync() post-processes the
Tile-scheduled module accordingly.
"""

import os
import numpy as np
import ml_dtypes
from contextlib import ExitStack

import concourse.bass as bass
import concourse.mybir as mybir
import concourse.tile as tile
from concourse.bass_utils import run_bass_kernel_spmd

B, C, P, F = 4096, 200, 32, 512
CP = C * P                  # 6400 prototypes
ALPHA, EPS = 5.0, 1e-8
N_CORES = 8
BLOC = B // N_CORES         # 512 rows per core
BB = BLOC // 128            # 4 row-blocks of 128
FC = F // 128               # 4 contraction chunks
T = 4                       # prototype planes per superblock
NSB = P // T                # 8 superblocks
SBW = C * T                 # 800 columns per superblock
PSUM_BUFS = int(os.environ.get("KPSB", "4"))  # psum pool tiles
PSW = int(os.environ.get("KPSW", "1024"))     # psum tile width (fp32 cols)
RECENTER = 512.0            # delta = RECENTER - |w|^2 rides the fold rows

_BF16 = mybir.dt.bfloat16
_F32 = mybir.dt.float32
_F8 = mybir.dt.float8e4

NROWS = 20                  # slot rows per row-block shipped to the host


def _emit(ctx, tc_ctx, io):
    """Superblock-pair schedule; see module docstring."""
    nc = tc_ctx.nc
    singles = ctx.enter_context(tc_ctx.tile_pool(name="singles", bufs=1))
    psum = ctx.enter_context(tc_ctx.tile_pool(name="psum", bufs=PSUM_BUFS,
                                              space="PSUM"))

    # weights laid out superblock-major so every DMA chunk is contiguous
    # per partition (one fat descriptor instead of FC strided stripes)
    wt_t = singles.tile([128, NSB, FC, SBW], _F8)
    xt_t = singles.tile([128, FC, BLOC], _F8)    # X^T (+ fold const rows)
    acc = singles.tile([128, BB, NROWS, C], _BF16)  # per-row-block slot rows
    wrm = singles.tile([128, 2, 128], _F8)       # HAM warmup scratch

    pm = mybir.MatmulPerfMode.DoubleRow

    # Input DMA dispatches cost ~0.65us of dispatcher-engine time each; put
    # wt superblock 0 first (it gates the first real matmul) and xt on the
    # scalar engine so the two leading feeds start in parallel.
    wt_in = io["wt"][:, :].rearrange("p (s fc j) -> p s fc j", s=NSB, fc=FC)
    nc.sync.dma_start(out=wt_t[:, 0:1], in_=wt_in[:, 0:1])
    nc.scalar.dma_start(
        out=xt_t[:, :, :],
        in_=io["xt"][:, :].rearrange("p (fc b) -> p fc b", fc=FC))
    for s0, s1 in ((1, 2), (2, 3), (3, 4), (4, 6), (6, 8)):
        nc.sync.dma_start(out=wt_t[:, s0:s1], in_=wt_in[:, s0:s1])

    # HAM warmup: the PE clock-gate sits at 1.2GHz until ~3.4us of sustained
    # matmul activity.  Spin dependency-free garbage matmuls during the DMA
    # head so the real stream runs at 2.4GHz from its first instruction.
    n_warm = int(os.environ.get("KWARM", "36"))
    if n_warm:
        nc.vector.memset(wrm[:, :, :], 0.0)
        pw = psum.tile([128, PSW], _F32, tag="ps")
        for _ in range(n_warm):
            nc.tensor.matmul(pw[:, 0:128], wrm[:, :, 0:128], wrm[:, :, :],
                             start=True, stop=True, perf_mode=pm,
                             skip_group_check=True)

    NQ = NSB // 2            # 4 superblock pairs
    order = [(q, bb) for q in range(NQ - 1) for bb in range(BB)]
    order += [(NQ - 1, bb) for bb in range(BB - 1, -1, -1)]

    # seed-only consumer plan: every drain seeds its own slot rows in acc
    # (no accumulate chains, no staging, no TT merges at all), host folds
    # the NROWS rows per row-block.  Rows are grouped per pair q (5 rows:
    # ACT's 4 + TR's 1) and each group ships as soon as its pair's two
    # drains finish, so the output stream overlaps the matmul stream and
    # only the last pair's 0.26MB rides the tail:
    #   h0 (even sb):  ACT copies psum straight into rows 5q:5q+4
    #   h1 (odd sb):   TR over the 4 planes seeds row 5q+4
    def consume0(q, ps, bb):     # h0 chain: ACT seeds 4 rows directly
        psv = ps[:, 0:SBW].rearrange("p (c t) -> p t c", t=T)
        nc.scalar.activation(acc[:, bb, 5 * q:5 * q + T, :], psv,
                             mybir.ActivationFunctionType.Copy)

    def consume1(q, ps, bb):     # h1 chain: direct DVE reduce, frees psum
        rin = ps[:, 0:SBW].rearrange("p (c t) -> p c t", t=T)
        nc.vector.tensor_reduce(
            out=acc[:, bb, 5 * q + 4, :], in_=rin,
            axis=mybir.AxisListType.X, op=mybir.AluOpType.max)

    for q, bb in order:
        s0, s1 = 2 * q, 2 * q + 1
        ps0 = psum.tile([128, PSW], _F32, tag="ps")
        ps1 = psum.tile([128, PSW], _F32, tag="ps")
        rows = slice(bb * 128, (bb + 1) * 128)
        for pi in range(2):
            for s, ps in ((s0, ps0), (s1, ps1)):
                for p0, p1 in ((0, 512), (512, SBW)):
                    nc.tensor.matmul(
                        ps[:, p0:p1],
                        xt_t[:, 2 * pi:2 * pi + 2, rows],
                        wt_t[:, s, 2 * pi:2 * pi + 2, p0:p1],
                        start=(pi == 0), stop=(pi == 1), perf_mode=pm)

        consume1(q, ps0, bb)   # TR drains the pair's FIRST psum tile: the
        consume0(q, ps1, bb)   # next-next pair's lead MMs see a fast free

        r0 = bb * NROWS * C + 5 * q * C
        nc.sync.dma_start(
            out=io["accq"][:, r0:r0 + 5 * C],
            in_=acc[:, bb, 5 * q:5 * q + 5, :].rearrange("p r c -> p (r c)"))


_RANGE_CLEAR_OPCODE = 176


def _dedup_ldweights(nc):
    """Drop InstLdweights whose stationary AP matches the previous Ldweights
    on the PE stream (bass lowers every matmul into Ldweights + Matmult;
    consecutive matmuls sharing lhsT reload identical weights for nothing --
    each reload is ~126ns of PE time plus sequencer pressure).

    A dropped Ldweights' waits and sem updates migrate to the next kept PE
    instruction so cross-engine vector clocks stay intact.
    """
    for fn in nc.m.functions:
        for blk in fn.blocks:
            last_key = None
            pend_waits, pend_ups = [], []
            out = []
            for ins in blk.instructions:
                tn = type(ins).__name__
                if getattr(ins, "engine", None) != mybir.EngineType.PE:
                    out.append(ins)
                    continue
                if tn == "InstLdweights":
                    key = (str(ins.ins[0]), str(ins.perf_mode),
                           str(getattr(ins, "tile_position", None)))
                    si = ins.sync_info
                    if key == last_key:
                        if si is not None:
                            pend_waits += list(si.on_wait)
                            pend_ups += list(si.on_update)
                        continue
                    last_key = key
                if pend_waits or pend_ups:
                    si = ins.sync_info
                    waits = list(si.on_wait) if si else []
                    ups = list(si.on_update) if si else []
                    # merge duplicate sem-inc updates by summing values
                    for u in pend_ups:
                        for v in ups:
                            if (u.sync_type == v.sync_type
                                    and getattr(u, "id", None) == getattr(v, "id", None)
                                    and u.update_mode == v.update_mode == "sem-inc"):
                                v.update_value += u.update_value
                                break
                        else:
                            ups.append(u)
                    ins.sync_info = mybir.SyncInfo(
                        on_wait=pend_waits + waits, on_update=ups)
                    pend_waits, pend_ups = [], []
                out.append(ins)
            assert not pend_waits and not pend_ups, (
                "dangling sync from dropped trailing Ldweights")
            if hasattr(blk, "set_instructions"):
                blk.set_instructions(out)
            else:
                blk.instructions = out


def _legalize_sync(nc):
    """Adapt the Tile-scheduled module to this container's walrus build:

    1. TPB instruction encodings here accept at most ONE inline sync wait
       ("Too many sync wait commands"), so hoist extra waits into standalone
       single-wait EventSemaphore instructions on the same engine.
    2. The tail EVENT_SEMAPHORE_RANGE_CLEAR InstISA is rejected ("ISA wrong
       length"); replace it with per-semaphore write-0 updates.
    """
    wid = [0]
    reset_done = set()   # sem ids already cleared once (drain + range-clear
                         # both cover the same range -- emit each id once)
    _eng_rr = [mybir.EngineType.Pool, mybir.EngineType.SP,
               mybir.EngineType.DVE, mybir.EngineType.Activation,
               mybir.EngineType.PE]

    def mk(engine, waits, updates):
        ev = mybir.InstEventSemaphore(name=f"WSPLIT-{wid[0]}")
        wid[0] += 1
        ev.engine = engine
        ev.sync_info = mybir.SyncInfo(on_wait=waits, on_update=updates)
        return ev

    for fn in nc.m.functions:
        for blk in fn.blocks:
            out = []
            for ins in blk.instructions:
                si = ins.sync_info
                if si is not None and len(si.on_wait) > 1:
                    for w in si.on_wait[:-1]:
                        out.append(mk(ins.engine, [w], []))
                    ins.sync_info = mybir.SyncInfo(
                        on_wait=[si.on_wait[-1]], on_update=list(si.on_update))
                if (type(ins).__name__ == "InstDrain"
                        and getattr(ins, "is_reset_sema", False)):
                    first = ins.reset_range_start
                    last = ins.reset_range_stop - 1
                    ins.is_reset_sema = False
                    ups = [mybir.SyncUpdate(sync_type="semaphore", id=s,
                                            update_mode="sem-wr-imm",
                                            update_value=0)
                           for s in range(first, last + 1)
                           if s not in reset_done]
                    reset_done.update(range(first, last + 1))
                    out.append(ins)
                    # all engines are quiescent behind the exit barrier at
                    # this point; spread the clears across them
                    for k, u in enumerate(ups):
                        out.append(mk(_eng_rr[k % len(_eng_rr)], [], [u]))
                    continue
                if (type(ins).__name__ == "InstISA"
                        and getattr(ins, "isa_opcode", None) == _RANGE_CLEAR_OPCODE):
                    import re as _re
                    m = _re.search(r"range_first=(\d+) range_last=(\d+)", str(ins))
                    first, last = int(m.group(1)), int(m.group(2))
                    ups = [mybir.SyncUpdate(sync_type="semaphore", id=s,
                                            update_mode="sem-wr-imm",
                                            update_value=0)
                           for s in range(first, last + 1)
                           if s not in reset_done]
                    reset_done.update(range(first, last + 1))
                    for u in ups:
                        out.append(mk(ins.engine, [], [u]))
                    continue
                out.append(ins)
            blk.set_instructions(out) if hasattr(blk, "set_instructions") else None
            if not hasattr(blk, "set_instructions"):
                blk.instructions = out


_NC_CACHE = {}


def build_nc(legalize=True, reps=1, loop=0):
    key = (legalize, reps, loop)
    if key in _NC_CACHE:
        return _NC_CACHE[key]
    nc = bass.Bass(enable_partition_id=(os.environ.get("KPID", "0") == "1"))
    io = {
        "wt": nc.declare_dram_parameter("wt", [128, FC * CP], _F8,
                                        isOutput=False),
        "xt": nc.declare_dram_parameter("xt", [128, FC * BLOC], _F8,
                                        isOutput=False),
        "accq": nc.declare_dram_parameter("accq", [128, BB * NROWS * C],
                                          _BF16, isOutput=True),
    }
    with tile.TileContext(nc) as tc_ctx:
        if loop:
            with tc_ctx.For_i(0, loop, 1):
                with ExitStack() as ctx:
                    _emit(ctx, tc_ctx, io)
        else:
            for _ in range(reps):
                with ExitStack() as ctx:
                    _emit(ctx, tc_ctx, io)
    if os.environ.get("KLDWDEDUP", "1") == "1":
        _dedup_ldweights(nc)
    if legalize:
        _legalize_sync(nc)
    _NC_CACHE[key] = nc
    return nc


def _colperm():
    """Device column order: col = s*SBW + c*T + t  <->  proto p = s*T + t."""
    s = np.arange(NSB)[:, None, None]
    c = np.arange(C)[None, :, None]
    t = np.arange(T)[None, None, :]
    return (c * P + s * T + t).reshape(-1)   # j index per device column


def make_in_maps(outputs, clusters, target_classes):
    X = np.asarray(outputs, dtype=np.float32)
    W = np.asarray(clusters, dtype=np.float32).reshape(CP, F)
    tcl = np.asarray(target_classes).astype(np.int64)

    w2b = (2.0 * W).astype(ml_dtypes.float8_e4m3)         # [CP, F]
    wf = w2b.astype(np.float32) * 0.5                     # W the device sees
    delta = (RECENTER - np.sum(wf * wf, axis=1))          # [CP]
    u = (delta / 64.0).astype(ml_dtypes.float8_e4m3)
    v = (delta - 64.0 * u.astype(np.float32)).astype(ml_dtypes.float8_e4m3)

    perm = _colperm()
    wcols = w2b.T[:, perm]                                # [F, CPdev]
    # wt host image matches the SBUF tile exactly: [128, NSB, FC, SBW]
    # (superblock-major so each superblock's bytes are contiguous/partition)
    wt = np.ascontiguousarray(
        wcols.reshape(FC, 128, NSB, SBW).transpose(1, 2, 0, 3))
    # bias rows replace features 510/511 (partitions 126/127 of fc=3)
    wt[126, :, 3, :] = u[perm].reshape(NSB, SBW)
    wt[127, :, 3, :] = v[perm].reshape(NSB, SBW)
    wt = wt.reshape(128, FC * CP)

    in_maps = []
    for cidx in range(N_CORES):
        xs = X[cidx * BLOC:(cidx + 1) * BLOC]             # [BLOC, F]
        xq = xs.astype(ml_dtypes.float8_e4m3)
        xt = np.ascontiguousarray(xq.T).reshape(FC, 128, BLOC)
        xt = np.ascontiguousarray(np.transpose(xt, (1, 0, 2)))  # [128,FC,BLOC]
        xt[126, 3, :] = np.float32(64.0)
        xt[127, 3, :] = np.float32(1.0)
        m = {"wt": wt, "xt": xt.reshape(128, FC * BLOC)}
        in_maps.append(m)
    return in_maps, X


def host_rows(results, target_classes):
    """Fold the shipped accumulators and select vt/vw per row (host side).

    Device accq[p, bb, slot, c] holds two partial per-class maxes of
    nq + RECENTER; row b = bb*128 + p of that core's shard.
    """
    tcl = np.asarray(target_classes).astype(np.int64)
    vt = np.empty(B, np.float64)
    vw = np.empty(B, np.float64)
    rows = np.arange(BLOC)
    for cidx, r in enumerate(results):
        a = r["accq"].astype(np.float32).reshape(128, BB, NROWS, C)
        maxq = a.max(axis=2)                       # [128, BB, C]
        maxq = maxq.transpose(1, 0, 2).reshape(BLOC, C).astype(np.float64)
        tc = tcl[cidx * BLOC:(cidx + 1) * BLOC]
        vt[cidx * BLOC:(cidx + 1) * BLOC] = maxq[rows, tc]
        m2 = maxq.copy()
        m2[rows, tc] = -np.inf
        vw[cidx * BLOC:(cidx + 1) * BLOC] = m2.max(axis=1)
    return vt - RECENTER, vw - RECENTER


def combine(results, X, target_classes):
    vt, vw = host_rows(results, target_classes)
    sx2 = float((X.astype(np.float64) ** 2).sum())
    tl = (sx2 - vt.sum()) / (B * F)
    ntl = (sx2 - vw.sum()) / (B * F)
    return np.float32((1.0 - ALPHA) * tl + ALPHA / (ntl + EPS))


def kernel(outputs, clusters, target_classes):
    nc = build_nc()
    in_maps, X = make_in_maps(outputs, clusters, target_classes)
    res = run_bass_kernel_spmd(nc, in_maps, core_ids=list(range(N_CORES))).results
    return combine(res, X, target_classes)

